# revision 12
# baseline (speedup 1.0000x reference)
"""Trainium2 Bass kernel for nn_BasisPooling.

The reference computes, per 2x2 non-overlapping patch (K=4, kernel-ordered
p0=x[2i,2j], p1=x[2i,2j+1], p2=x[2i+1,2j], p3=x[2i+1,2j+1]):

    scores[d,k] = patch_var + pos_bias[k] * offset[d]
    weights     = softmax_k(scores / T)
    out[d]      = sum_k weights[d,k] * p_k

patch_var does not depend on k, so it cancels inside the softmax: the
weights are data-independent constants w[d,k] = softmax_k(pos_bias[k] *
offset[d] / T).  The whole module is therefore two fixed 4-tap blends of
each 2x2 patch -- a purely memory-bound strided map:

    out[b, 2c+d, i, j] = sum_k w[d,k] * p_k(b, c, i, j)

Mapping: pure data parallel over batch (32 -> 4 per core x 8 cores).
Per core: channels (128) live on the SBUF partition dim; the image is
processed in half-example chunks of 56 input rows.  DVE evaluates each
output with a 3-op Horner chain (scalar_tensor_tensor: out = (in0 *
s) + in1), anchored on an ACT-prescaled term q_d = w[d,anchor] * p_anchor.
"""

import numpy as np

import concourse.bacc as bacc
import concourse.mybir as mybir
import concourse.tile as tile
from concourse.bass_utils import run_bass_kernel_spmd

N_CORES = 8
B_FULL = 32
B = B_FULL // N_CORES  # examples per core
C = 128
H = W = 112
OH = OW = 56
RH = 56          # input rows per chunk
OCH = RH // 2    # output rows per chunk
NCHUNK = H // RH
F32 = mybir.dt.float32
MULT = mybir.AluOpType.mult
ADD = mybir.AluOpType.add
COPY = mybir.ActivationFunctionType.Copy


def _softmax_weights(temperature: float) -> np.ndarray:
    """w[d, k] = softmax_k(pos_bias[k] * offset[d] / T), matching reference."""
    pos = np.linspace(0.0, 1.0, 4, dtype=np.float64)
    offs = np.linspace(-0.5, 0.5, 2, dtype=np.float64)
    logits = pos[None, :] * offs[:, None] / np.float64(temperature)
    e = np.exp(logits - logits.max(axis=1, keepdims=True))
    return e / e.sum(axis=1, keepdims=True)  # [2, 4]


def _build(w: np.ndarray, repeat: int = 1, mode: str = "full"):
    # mode: "full" | "dma" (chunked DMAs, no compute) | "dma2" (full-example
    # DMAs) — timing diagnostics; only "full" produces correct results.
    # repeat > 1 repeats the whole body (idempotent) for slope-based timing.
    nc = bacc.Bacc("TRN2", target_bir_lowering=False, debug=False)
    x = nc.dram_tensor("x", [B, C, H, W], F32, kind="ExternalInput")
    y = nc.dram_tensor("y", [B, 2 * C, OH, OW], F32, kind="ExternalOutput")
    yv = y.rearrange("b (c d) h w -> b c d h w", d=2)  # [B, 128, 2, 56, 56]

    with tile.TileContext(nc) as tc:
        with (
            tc.tile_pool(name="io", bufs=3) as iop,
            tc.tile_pool(name="tmp", bufs=2) as tmpp,
        ):
            if mode == "dma2":
                # full-example DMA pattern: 6.4 MB loads, one fully
                # contiguous 3.2 MB store per example
                out_dummy = iop.tile([C, 2, OH, OW], F32, tag="ydummy", bufs=1)
                nc.vector.memset(out_dummy[:], 0.0)
                for b in [b for _ in range(repeat) for b in range(B)]:
                    xin = iop.tile([C, H, W], F32, tag="xin", bufs=3)
                    nc.scalar.dma_start(out=xin[:], in_=x[b])
                    nc.sync.dma_start(out=yv[b], in_=out_dummy[:])
            out_dummy = None
            if mode == "dma":
                out_dummy = iop.tile([C, 2, OCH, OW], F32, tag="ydummy", bufs=1)
                nc.vector.memset(out_dummy[:], 0.0)
            chunks = [] if mode == "dma2" else [
                (b, h)
                for _ in range(repeat)
                for b in range(B)
                for h in range(NCHUNK)
            ]
            for b, half in chunks:
                h0 = half * RH
                i0 = half * OCH
                xin = iop.tile([C, RH, W], F32, tag="xin")
                # HWDGE load on ACT's ring; stores ride SP's ring so
                # neither queue head-of-line-blocks the other.
                nc.scalar.dma_start(out=xin[:], in_=x[b, :, h0 : h0 + RH, :])

                if mode == "dma":
                    nc.sync.dma_start(
                        out=yv[b, :, :, i0 : i0 + OCH, :], in_=out_dummy[:]
                    )
                    continue

                p0 = xin[:, 0::2, 0::2]
                p1 = xin[:, 0::2, 1::2]
                p2 = xin[:, 1::2, 0::2]
                p3 = xin[:, 1::2, 1::2]

                out_t = iop.tile([C, 2, OCH, OW], F32, tag="yout")

                # d = 0: out0 = w00*p0 + w01*p1 + w02*p2 + w03*p3
                q0 = tmpp.tile([C, OCH, OW], F32, tag="q0")
                nc.scalar.activation(q0[:], p0, COPY, scale=float(w[0, 0]))
                h1 = tmpp.tile([C, OCH, OW], F32, tag="h1")
                nc.vector.scalar_tensor_tensor(
                    h1[:], p3, float(w[0, 3] / w[0, 2]), p2, op0=MULT, op1=ADD
                )
                h2 = tmpp.tile([C, OCH, OW], F32, tag="h2")
                nc.vector.scalar_tensor_tensor(
                    h2[:], h1[:], float(w[0, 2] / w[0, 1]), p1, op0=MULT, op1=ADD
                )
                nc.vector.scalar_tensor_tensor(
                    out_t[:, 0], h2[:], float(w[0, 1]), q0[:], op0=MULT, op1=ADD
                )

                # d = 1: out1 = w10*p0 + w11*p1 + w12*p2 + w13*p3
                q1 = tmpp.tile([C, OCH, OW], F32, tag="q1")
                nc.scalar.activation(q1[:], p3, COPY, scale=float(w[1, 3]))
                g1 = tmpp.tile([C, OCH, OW], F32, tag="g1")
                nc.vector.scalar_tensor_tensor(
                    g1[:], p0, float(w[1, 0] / w[1, 1]), p1, op0=MULT, op1=ADD
                )
                g2 = tmpp.tile([C, OCH, OW], F32, tag="g2")
                nc.vector.scalar_tensor_tensor(
                    g2[:], g1[:], float(w[1, 1] / w[1, 2]), p2, op0=MULT, op1=ADD
                )
                nc.vector.scalar_tensor_tensor(
                    out_t[:, 1], g2[:], float(w[1, 2]), q1[:], op0=MULT, op1=ADD
                )

                nc.sync.dma_start(
                    out=yv[b, :, :, i0 : i0 + OCH, :], in_=out_t[:]
                )

    nc.compile()
    return nc


_CACHE: dict[float, object] = {}


def kernel(x: np.ndarray, temperature: np.ndarray) -> np.ndarray:
    t = float(np.asarray(temperature).reshape(-1)[0])
    w = _softmax_weights(t)
    nc = _CACHE.get(t)
    if nc is None:
        nc = _build(w)
        _CACHE[t] = nc

    x = np.ascontiguousarray(np.asarray(x, dtype=np.float32))
    in_maps = [
        {"x": np.ascontiguousarray(x[c * B : (c + 1) * B])} for c in range(N_CORES)
    ]
    res = run_bass_kernel_spmd(nc, in_maps, list(range(N_CORES)))
    return np.concatenate([r["y"] for r in res.results], axis=0)


# revision 14
# speedup vs baseline: 1.0197x; 1.0197x over previous
"""Trainium2 Bass kernel for nn_BasisPooling.

The reference computes, per 2x2 non-overlapping patch (K=4, kernel-ordered
p0=x[2i,2j], p1=x[2i,2j+1], p2=x[2i+1,2j], p3=x[2i+1,2j+1]):

    scores[d,k] = patch_var + pos_bias[k] * offset[d]
    weights     = softmax_k(scores / T)
    out[d]      = sum_k weights[d,k] * p_k

patch_var does not depend on k, so it cancels inside the softmax: the
weights are data-independent constants w[d,k] = softmax_k(pos_bias[k] *
offset[d] / T).  The whole module is therefore two fixed 4-tap blends of
each 2x2 patch -- a purely memory-bound strided map:

    out[b, 2c+d, i, j] = sum_k w[d,k] * p_k(b, c, i, j)

Mapping: pure data parallel over batch (32 -> 4 per core x 8 cores).
Per core: channels (128) live on the SBUF partition dim; the image is
processed in half-example chunks of 56 input rows.  DVE evaluates each
output with a 3-op Horner chain (scalar_tensor_tensor: out = (in0 *
s) + in1), anchored on an ACT-prescaled term q_d = w[d,anchor] * p_anchor.
"""

import numpy as np

import concourse.bacc as bacc
import concourse.mybir as mybir
import concourse.tile as tile
from concourse.bass_utils import run_bass_kernel_spmd

N_CORES = 8
B_FULL = 32
B = B_FULL // N_CORES  # examples per core
C = 128
H = W = 112
OH = OW = 56
RH = 56          # input rows per chunk
OCH = RH // 2    # output rows per chunk
NCHUNK = H // RH
F32 = mybir.dt.float32
MULT = mybir.AluOpType.mult
ADD = mybir.AluOpType.add
COPY = mybir.ActivationFunctionType.Copy


def _softmax_weights(temperature: float) -> np.ndarray:
    """w[d, k] = softmax_k(pos_bias[k] * offset[d] / T), matching reference."""
    pos = np.linspace(0.0, 1.0, 4, dtype=np.float64)
    offs = np.linspace(-0.5, 0.5, 2, dtype=np.float64)
    logits = pos[None, :] * offs[:, None] / np.float64(temperature)
    e = np.exp(logits - logits.max(axis=1, keepdims=True))
    return e / e.sum(axis=1, keepdims=True)  # [2, 4]


def _build(w: np.ndarray, repeat: int = 1, mode: str = "full"):
    # mode: "full" | "dma" (chunked DMAs, no compute) | "dma2" (full-example
    # DMAs) — timing diagnostics; only "full" produces correct results.
    # repeat > 1 repeats the whole body (idempotent) for slope-based timing.
    nc = bacc.Bacc("TRN2", target_bir_lowering=False, debug=False)
    x = nc.dram_tensor("x", [B, C, H, W], F32, kind="ExternalInput")
    y = nc.dram_tensor("y", [B, 2 * C, OH, OW], F32, kind="ExternalOutput")
    yv = y.rearrange("b (c d) h w -> b c d h w", d=2)  # [B, 128, 2, 56, 56]

    with tile.TileContext(nc) as tc:
        with (
            tc.tile_pool(name="io", bufs=3) as iop,
            tc.tile_pool(name="tmp", bufs=2) as tmpp,
        ):
            if mode == "dma2":
                # full-example DMA pattern: 6.4 MB loads, one fully
                # contiguous 3.2 MB store per example
                out_dummy = iop.tile([C, 2, OH, OW], F32, tag="ydummy", bufs=1)
                nc.vector.memset(out_dummy[:], 0.0)
                for b in [b for _ in range(repeat) for b in range(B)]:
                    xin = iop.tile([C, H, W], F32, tag="xin", bufs=3)
                    nc.scalar.dma_start(out=xin[:], in_=x[b])
                    nc.sync.dma_start(out=yv[b], in_=out_dummy[:])
            out_dummy = None
            if mode == "dma":
                out_dummy = iop.tile([C, 2, OCH, OW], F32, tag="ydummy", bufs=1)
                nc.vector.memset(out_dummy[:], 0.0)
            chunks = [] if mode == "dma2" else [
                (b, h)
                for _ in range(repeat)
                for b in range(B)
                for h in range(NCHUNK)
            ]
            for b, half in chunks:
                h0 = half * RH
                i0 = half * OCH
                xin = iop.tile([C, RH, W], F32, tag="xin")
                # HWDGE load on ACT's ring; stores ride SP's ring so
                # neither queue head-of-line-blocks the other.
                nc.scalar.dma_start(out=xin[:], in_=x[b, :, h0 : h0 + RH, :])

                if mode == "dma":
                    nc.sync.dma_start(
                        out=yv[b, :, :, i0 : i0 + OCH, :], in_=out_dummy[:]
                    )
                    continue

                p0 = xin[:, 0::2, 0::2]
                p1 = xin[:, 0::2, 1::2]
                p2 = xin[:, 1::2, 0::2]
                p3 = xin[:, 1::2, 1::2]

                out_t = iop.tile([C, 2, OCH, OW], F32, tag="yout")

                # d = 0: out0 = w00*p0 + w01*p1 + w02*p2 + w03*p3
                q0 = tmpp.tile([C, OCH, OW], F32, tag="q0")
                nc.scalar.activation(q0[:], p0, COPY, scale=float(w[0, 0]))
                h1 = tmpp.tile([C, OCH, OW], F32, tag="h1")
                nc.vector.scalar_tensor_tensor(
                    h1[:], p3, float(w[0, 3] / w[0, 2]), p2, op0=MULT, op1=ADD
                )
                h2 = tmpp.tile([C, OCH, OW], F32, tag="h2")
                nc.vector.scalar_tensor_tensor(
                    h2[:], h1[:], float(w[0, 2] / w[0, 1]), p1, op0=MULT, op1=ADD
                )
                nc.vector.scalar_tensor_tensor(
                    out_t[:, 0], h2[:], float(w[0, 1]), q0[:], op0=MULT, op1=ADD
                )

                # d = 1: out1 = w10*p0 + w11*p1 + w12*p2 + w13*p3
                q1 = tmpp.tile([C, OCH, OW], F32, tag="q1")
                nc.scalar.activation(q1[:], p3, COPY, scale=float(w[1, 3]))
                g1 = tmpp.tile([C, OCH, OW], F32, tag="g1")
                nc.vector.scalar_tensor_tensor(
                    g1[:], p0, float(w[1, 0] / w[1, 1]), p1, op0=MULT, op1=ADD
                )
                g2 = tmpp.tile([C, OCH, OW], F32, tag="g2")
                nc.vector.scalar_tensor_tensor(
                    g2[:], g1[:], float(w[1, 1] / w[1, 2]), p2, op0=MULT, op1=ADD
                )
                nc.vector.scalar_tensor_tensor(
                    out_t[:, 1], g2[:], float(w[1, 2]), q1[:], op0=MULT, op1=ADD
                )

                nc.sync.dma_start(
                    out=yv[b, :, :, i0 : i0 + OCH, :], in_=out_t[:]
                )

    nc.compile()
    return nc


_CACHE: dict[float, object] = {}


def kernel(x: np.ndarray, temperature: np.ndarray) -> np.ndarray:
    t = float(np.asarray(temperature).reshape(-1)[0])
    w = _softmax_weights(t)
    nc = _CACHE.get(t)
    if nc is None:
        nc = _build(w)
        _CACHE[t] = nc

    x = np.ascontiguousarray(np.asarray(x, dtype=np.float32))
    in_maps = [
        {"x": np.ascontiguousarray(x[c * B : (c + 1) * B])} for c in range(N_CORES)
    ]
    res = run_bass_kernel_spmd(nc, in_maps, list(range(N_CORES)))
    return np.concatenate([r["y"] for r in res.results], axis=0)
